# revision 1
# baseline (speedup 1.0000x reference)
"""Multi-head attention (dense_transformer) on 8 TRN2 NeuronCores.

Sharding: 2-way data parallel over batch x 4-way tensor parallel over heads.
Core c handles batch b=c//4 and heads {4g..4g+3} where g=c%4 (4 heads, 256
channels per core; channels of head h are qw columns {hd*16+h}).

Per core:
  phase 1: Q^T/K^T/V^T projections ([ch, s] layout, fp32 PE matmuls)
  phase 2: causal flash-style attention per head: scores [q,k] in PSUM (fp32),
           rowmax -> exp(bias=-max) on ACT (accum row sums) -> normalize by
           1/sum -> PE-transpose normalized attn -> O^T = V^T@attnT
           (two heads col-packed into one PSUM tile via tile_position)
  phase 3: AllGather O^T across the 4 cores of the batch -> out-proj column
           slice (out^T = ow_perm^T @ merged^T, float32r) -> DMA out.

dtype notes (measured on HW): float32r matmul = ~11-bit mantissa; declaring a
DRAM tensor float32r poisons even "fp32" matmuls on it (bitcast doesn't help),
so precision-critical tensors (x, qw, kw, vw) are declared float32 and the
Q/K/scores path runs true fp32. The out-projection path (wo, O^T, allgather)
stays float32r: its error contribution is linear and tiny.
"""
import sys

sys.path.insert(0, "/opt/trn_rl_repo")

import numpy as np

import concourse.bass as bass
import concourse.mybir as mybir
import concourse.tile as tile
from concourse import bacc
from concourse.bass_utils import run_bass_kernel_spmd
from concourse.masks import make_causal_mask, make_identity

# ---- problem constants (hardcoded per harness contract) ----
B, S, D, HEADS = 2, 2048, 1024, 16
N_CORES = 8
GROUPS = 4                 # head-groups == cores per batch
HPC = HEADS // GROUPS      # heads per core (4)
HD = D // HEADS            # 64
CPC = HPC * HD             # channels per core (256)
P = 128
NCC = CPC // P             # col chunks per core (2)
DCH = D // P               # contraction chunks (8)

f32 = mybir.dt.float32
f32r = mybir.dt.float32r
bf16 = mybir.dt.bfloat16

AX = mybir.AxisListType
EXP = mybir.ActivationFunctionType.Exp

DEFAULT_CFG = dict(
    s=S,
    attn_dt="bf16",    # "bf16" | "f32" for attn weights / V / AV matmul
    gw=512,            # AV group width (q columns per O^T psum tile)
    fold_norm=False,   # diag(rec)-in-transpose: rejected by HW (perm matrix only)
)


def build_nc(s=S, attn_dt="bf16", gw=256, fold_norm=True, dbg=False):
    assert s % 512 == 0
    NQI = s // P           # q chunks of 128
    NQB = s // 512         # 512-wide q blocks (projection / scores k blocks)
    QPG = gw // P          # q chunks per AV group
    NGRP = s // gw         # AV groups

    att_dt = {"bf16": bf16, "f32": f32}[attn_dt]
    del attn_dt

    nc = bacc.Bacc("TRN2", target_bir_lowering=False, debug=False,
                   num_devices=N_CORES)
    xT = nc.dram_tensor("xT", [D, s], f32, kind="ExternalInput").ap()
    wq = nc.dram_tensor("wq", [D, CPC], f32, kind="ExternalInput").ap()
    wk = nc.dram_tensor("wk", [D, CPC], f32, kind="ExternalInput").ap()
    wv = nc.dram_tensor("wv", [D, CPC], f32, kind="ExternalInput").ap()
    wo = nc.dram_tensor("wo", [D, CPC], f32r, kind="ExternalInput").ap()
    outT = nc.dram_tensor("outT", [NCC, P, s], f32, kind="ExternalOutput").ap()
    dbg_t = {}
    if dbg:
        for nm, shp in (("QT", [P, NCC, s]), ("KT", [P, NCC, s]),
                        ("Vsb", [P, s // P, CPC]), ("OT", [P, NCC, s]),
                        ("at0", [P, s]), ("sc0", [P, 4, 512]),
                        ("atT0", [P, s // P, gw])):
            dbg_t[nm] = nc.dram_tensor("dbg_" + nm, shp, f32,
                                       kind="ExternalOutput").ap()

    with tile.TileContext(nc) as tc:
        with (
            tc.tile_pool(name="cpool", bufs=1) as cpool,
            tc.tile_pool(name="wpool", bufs=1) as wpool,
            tc.tile_pool(name="big", bufs=1) as big,
            tc.tile_pool(name="xs", bufs=6) as xs,
            tc.tile_pool(name="apool", bufs=3) as apool,
            tc.tile_pool(name="atp", bufs=1) as atp,
            tc.tile_pool(name="stat", bufs=8) as stat,
            tc.tile_pool(name="scb", bufs=6) as scbp,
            tc.tile_pool(name="ms", bufs=6) as ms,
            tc.tile_pool(name="op", bufs=3) as op,
            tc.tile_pool(name="dram", bufs=1, space="DRAM") as dpool,
        ):
            NH = 2 if s >= 1024 else 1       # AllGather halves per head pair
            HW2 = s // NH
            ag_in = {(hp, hf): dpool.tile([P, HW2], f32r,
                                          tag=f"agin{hp}{hf}", name=f"agin{hp}{hf}")
                     for hp in range(NCC) for hf in range(NH)}
            ag_out = {(hp, hf): dpool.tile([GROUPS, P, HW2], f32r,
                                           tag=f"agout{hp}{hf}",
                                           name=f"agout{hp}{hf}")
                      for hp in range(NCC) for hf in range(NH)}

            Wmask = cpool.tile([P, P], f32, tag="Wmask")
            make_causal_mask(nc, Wmask[:], mask_val=-1e10)
            ident = cpool.tile([P, P], att_dt, tag="ident")
            make_identity(nc, ident[:])

            wo_sb = wpool.tile([P, DCH, CPC], f32r, tag="wo")
            nc.sync.dma_start(wo_sb[:], wo.rearrange("(o p) c -> p o c", p=P))
            wsplit = {}
            for nm, wdr in (("q", wq), ("k", wk), ("v", wv)):
                wh = wpool.tile([P, DCH, CPC], bf16, tag=f"w{nm}h", name=f"w{nm}h")
                wl = wpool.tile([P, DCH, CPC], bf16, tag=f"w{nm}l", name=f"w{nm}l")
                wsplit[nm] = [wh, wl]
            with tc.tile_pool(name="wload", bufs=1) as wload:
                for nm, wdr in (("q", wq), ("k", wk), ("v", wv)):
                    wf = wload.tile([P, DCH, CPC], f32, tag="wf", name="wf")
                    nc.sync.dma_start(wf[:], wdr.rearrange("(o p) c -> p o c", p=P))
                    wh, wl = wsplit[nm]
                    nc.vector.tensor_copy(wh[:], wf[:])
                    nc.vector.tensor_tensor(wl[:], wf[:], wh[:],
                                            mybir.AluOpType.subtract)

            QTh = big.tile([P, NCC, s], bf16, tag="QTh")
            QTl = big.tile([P, NCC, s], bf16, tag="QTl")
            KTh = big.tile([P, NCC, s], bf16, tag="KTh")
            KTl = big.tile([P, NCC, s], bf16, tag="KTl")
            VTb = big.tile([P, NCC, s], att_dt, tag="VTb")
            Vsb = big.tile([P, s // P, CPC], att_dt, tag="Vsb")
            OT = big.tile([P, NCC, s], f32r, tag="OT")

            # ---------------- phase 1: projections (fp32) ----------------
            with tc.tile_pool(name="psp", bufs=2, space="PSUM") as psp:
                for qb in range(NQB):
                    accs = {}
                    for nm in ("q", "k", "v"):
                        for cc in range(NCC):
                            accs[nm, cc] = psp.tile([P, 512], f32,
                                                    tag=f"pp{nm}", name=f"pp{nm}{cc}")
                    for di in range(DCH):
                        xt = xs.tile([P, 512], f32, tag="xt", name="xt")
                        nc.sync.dma_start(
                            xt[:], xT[di * P:(di + 1) * P, qb * 512:(qb + 1) * 512])
                        xth = xs.tile([P, 512], bf16, tag="xth", name="xth")
                        xtl = xs.tile([P, 512], bf16, tag="xtl", name="xtl")
                        nc.vector.tensor_copy(xth[:], xt[:])
                        nc.vector.tensor_tensor(xtl[:], xt[:], xth[:],
                                                mybir.AluOpType.subtract)
                        for nm in ("q", "k", "v"):
                            wh, wl = wsplit[nm]
                            for cc in range(NCC):
                                csl = slice(cc * P, (cc + 1) * P)
                                terms = [(wh, xth), (wh, xtl), (wl, xth)]
                                for ti, (wt, xtt) in enumerate(terms):
                                    nc.tensor.matmul(
                                        accs[nm, cc][:], wt[:, di, csl], xtt[:],
                                        start=(di == 0 and ti == 0),
                                        stop=(di == DCH - 1 and ti == len(terms) - 1))
                    sl = slice(qb * 512, (qb + 1) * 512)
                    for cc in range(NCC):
                        for hi_t, lo_t, ps in ((QTh, QTl, accs["q", cc]),
                                               (KTh, KTl, accs["k", cc])):
                            nc.any.tensor_copy(hi_t[:, cc, sl], ps[:])
                            nc.vector.tensor_tensor(lo_t[:, cc, sl], ps[:],
                                                    hi_t[:, cc, sl],
                                                    mybir.AluOpType.subtract)
                        nc.any.tensor_copy(VTb[:, cc, sl], accs["v", cc][:])

            # ---------------- phase 2: attention ----------------
            with (
                tc.tile_pool(name="pssc", bufs=4, space="PSUM") as pssc,
                tc.tile_pool(name="pspt", bufs=3, space="PSUM") as pspt,
                tc.tile_pool(name="psot", bufs=1, space="PSUM") as psot,
            ):
                # V^T -> V (PE transposes)
                for cc in range(NCC):
                    for ki in range(s // P):
                        pt = pspt.tile([P, P], att_dt, tag="pt", name="ptv")
                        nc.tensor.transpose(pt[:], VTb[:, cc, ki * P:(ki + 1) * P],
                                            ident[:])
                        nc.any.tensor_copy(Vsb[:, ki, cc * P:(cc + 1) * P], pt[:])

                def _gather_half(hp, hf):
                    hsl2 = slice(hf * HW2, (hf + 1) * HW2)
                    nc.sync.dma_start(ag_in[hp, hf][:], OT[:, hp, hsl2])
                    nc.gpsimd.collective_compute(
                        "AllGather", mybir.AluOpType.bypass,
                        replica_groups=[[0, 1, 2, 3], [4, 5, 6, 7]],
                        ins=[ag_in[hp, hf][:]], outs=[ag_out[hp, hf][:]],
                    )

                for hp in range(NCC):          # head pair == col chunk
                    hp_grps = range(NGRP)
                    grp_half = HW2 // gw - 1 if NH == 2 else -1
                    for grp in hp_grps:
                        if grp == grp_half + 1 and grp_half >= 0:
                            _gather_half(hp, 0)
                        atT = {}
                        for h2 in range(2):
                            atT[h2] = atp.tile([P, s // P, gw], att_dt,
                                               tag=f"atT{h2}", name=f"atT{h2}")
                            # zero invalid (future-k) diag regions
                            for dk in range(1, QPG):
                                ki = grp * QPG + dk
                                nc.any.memset(atT[h2][:, ki, 0:dk * P], 0.0)
                        for r in range(QPG):
                            qi = grp * QPG + r
                            nkb = qi // 4 + 1
                            wlast = (qi % 4 + 1) * P
                            for h2 in range(2):
                                hsl = slice(h2 * 64, (h2 + 1) * 64)
                                qsl_h = QTh[hsl, hp, qi * P:(qi + 1) * P]
                                qsl_l = QTl[hsl, hp, qi * P:(qi + 1) * P]
                                sc_tiles = []
                                for j in range(nkb):
                                    wj = 512 if j < nkb - 1 else wlast
                                    st = pssc.tile([P, 512], f32, tag="sc", name="sc")
                                    sc_tiles.append((st, wj))
                                # term-outer so the stationary Q operand is
                                # reused across k-blocks (fewer LDWEIGHTS)
                                for ti, (qq, kside) in enumerate(
                                        ((qsl_h, KTh), (qsl_h, KTl), (qsl_l, KTh))):
                                    for j, (st, wj) in enumerate(sc_tiles):
                                        kk = kside[hsl, hp,
                                                   j * 512:j * 512 + wj]
                                        nc.tensor.matmul(st[:, :wj], qq, kk,
                                                         start=(ti == 0),
                                                         stop=(ti == 2))
                                # drain scores to SBUF so PSUM banks recycle
                                # fast and iterations pipeline on the PE
                                sc_sb = []
                                for j, (st, wj) in enumerate(sc_tiles):
                                    sb_t = scbp.tile([P, 512], f32, tag="scb",
                                                     name="scb")
                                    nc.any.tensor_copy(sb_t[:, :wj], st[:, :wj])
                                    sc_sb.append((sb_t, wj))
                                sc_tiles = sc_sb
                                # additive causal mask on the diagonal subtile
                                last, wl = sc_tiles[-1]
                                nc.vector.tensor_add(last[:, wl - P:wl],
                                                     last[:, wl - P:wl], Wmask[:])
                                # row stats
                                mc = stat.tile([P, 4], f32, tag="mc", name="mc")
                                for j, (st, wj) in enumerate(sc_tiles):
                                    nc.vector.reduce_max(mc[:, j:j + 1], st[:, :wj],
                                                         axis=AX.X)
                                negm = stat.tile([P, 1], f32, tag="negm", name="negm")
                                if nkb > 1:
                                    m = stat.tile([P, 1], f32, tag="m", name="m")
                                    nc.vector.reduce_max(m[:], mc[:, :nkb], axis=AX.X)
                                    nc.vector.tensor_scalar_mul(negm[:], m[:], -1.0)
                                else:
                                    nc.vector.tensor_scalar_mul(negm[:], mc[:, 0:1],
                                                                -1.0)
                                dump_this = (dbg and hp == 0 and h2 == 0
                                             and qi == min(8, NQI - 1))
                                if dump_this:
                                    for j, (st, wj) in enumerate(sc_tiles):
                                        dsc = stat.tile([P, 512], f32, tag="dsc",
                                                        name="dsc")
                                        nc.vector.tensor_copy(dsc[:, :wj], st[:, :wj])
                                        nc.sync.dma_start(dbg_t["sc0"][:, j, :wj],
                                                          dsc[:, :wj])
                                at = apool.tile([P, s], att_dt, tag="at", name="at")
                                sums = stat.tile([P, 4], f32, tag="sums", name="sums")
                                for j, (st, wj) in enumerate(sc_tiles):
                                    nc.scalar.activation(
                                        at[:, j * 512:j * 512 + wj], st[:, :wj], EXP,
                                        bias=negm[:], accum_out=sums[:, j:j + 1])
                                Ssum = stat.tile([P, 1], f32, tag="Ssum", name="Ssum")
                                nc.vector.reduce_sum(Ssum[:], sums[:, :nkb], axis=AX.X)
                                rec = stat.tile([P, 1], f32, tag="rec", name="rec")
                                nc.vector.reciprocal(rec[:], Ssum[:])
                                ktot = (qi + 1) * P
                                if fold_norm:
                                    # transpose by diag(rec) instead of identity:
                                    # normalizes during the PE transpose
                                    tid = stat.tile([P, P], att_dt, tag="tid",
                                                    name="tid")
                                    nc.vector.tensor_scalar_mul(tid[:], ident[:],
                                                                rec[:])
                                else:
                                    nc.any.tensor_scalar_mul(at[:, :ktot],
                                                             at[:, :ktot], rec[:])
                                    tid = ident
                                if dump_this:
                                    for ki2 in range(qi + 1):
                                        dat = op.tile([P, P], f32, tag="dat",
                                                      name="dat")
                                        nc.any.tensor_copy(
                                            dat[:], at[:, ki2 * P:(ki2 + 1) * P])
                                        nc.sync.dma_start(
                                            dbg_t["at0"][:, ki2 * P:(ki2 + 1) * P],
                                            dat[:])
                                for ki in range(qi + 1):
                                    pt = pspt.tile([P, P], att_dt, tag="pt",
                                                   name="pta")
                                    nc.tensor.transpose(
                                        pt[:], at[:, ki * P:(ki + 1) * P], tid[:])
                                    nc.any.tensor_copy(
                                        atT[h2][:, ki, r * P:(r + 1) * P], pt[:])
                        # AV for this group (two heads col-packed)
                        nch = grp * QPG + QPG
                        otp = psot.tile([P, gw], f32, tag="ot", name="otp")
                        for h2 in range(2):
                            vcols = slice(hp * P + h2 * 64, hp * P + (h2 + 1) * 64)
                            for ki in range(nch):
                                nc.tensor.matmul(
                                    otp[h2 * 64:(h2 + 1) * 64, :],
                                    Vsb[:, ki, vcols], atT[h2][:, ki, :],
                                    start=(ki == 0), stop=(ki == nch - 1),
                                    tile_position=(0, h2 * 64))
                        nc.any.tensor_copy(OT[:, hp, grp * gw:(grp + 1) * gw], otp[:])
                        if dbg and hp == 0 and grp == min(8, NQI - 1) // QPG:
                            for ki in range(s // P):
                                cv = op.tile([P, gw], f32, tag="cv3", name="cv3")
                                nc.any.tensor_copy(cv[:], atT[0][:, ki])
                                nc.sync.dma_start(dbg_t["atT0"][:, ki], cv[:])
                    if NH == 1:
                        _gather_half(hp, 0)
                    else:
                        _gather_half(hp, 1)

            if dbg:
                for nm, t in (("QT", QTh), ("KT", KTh), ("OT", OT)):
                    tv = t.bitcast(f32) if t.dtype != f32 else t
                    for cc in range(NCC):
                        for sb2 in range(s // 512):
                            cv = op.tile([P, 512], f32, tag="cv", name="cv")
                            nc.any.tensor_copy(cv[:],
                                               tv[:, cc, sb2 * 512:(sb2 + 1) * 512])
                            nc.sync.dma_start(
                                dbg_t[nm][:, cc, sb2 * 512:(sb2 + 1) * 512], cv[:])
                for ki in range(s // P):
                    cv = op.tile([P, CPC], f32, tag="cv2", name="cv2")
                    nc.any.tensor_copy(cv[:], Vsb[:, ki])
                    nc.sync.dma_start(dbg_t["Vsb"][:, ki], cv[:])

            # ------------- phase 3: AllGather + out-proj (f32r) -------------
            with tc.tile_pool(name="pso", bufs=2, space="PSUM") as pso:
                nhb = max(1, (s // 512) // NH)  # 512-blocks per half
                for sb_ in range(s // 512):
                    hf = sb_ // nhb if NH == 2 else 0
                    osl = slice(sb_ * 512 - hf * HW2,
                                (sb_ + 1) * 512 - hf * HW2)
                    ssl = slice(sb_ * 512, (sb_ + 1) * 512)
                    accs = [pso.tile([P, 512], f32, tag="po", name=f"po{occ}")
                            for occ in range(NCC)]
                    for mch in range(DCH):
                        g_, cc_ = mch // NCC, mch % NCC
                        mt = ms.tile([P, 512], f32r, tag="mt", name="mt")
                        nc.sync.dma_start(mt[:], ag_out[cc_, hf][g_, :, osl])
                        for occ in range(NCC):
                            nc.tensor.matmul(
                                accs[occ][:], wo_sb[:, mch, occ * P:(occ + 1) * P],
                                mt[:], start=(mch == 0), stop=(mch == DCH - 1))
                    for occ in range(NCC):
                        oo = op.tile([P, 512], f32, tag="oo", name="oo")
                        nc.any.tensor_copy(oo[:], accs[occ][:])
                        nc.sync.dma_start(outT[occ, :, ssl], oo[:])

    nc.compile()
    return nc


_NC_CACHE = {}


def get_nc(**cfg):
    key = tuple(sorted(cfg.items()))
    if key not in _NC_CACHE:
        _NC_CACHE[key] = build_nc(**cfg)
    return _NC_CACHE[key]


def _col_index(g):
    p = np.arange(CPC)
    return (p % HD) * HEADS + (HPC * g + p // HD)


def _ow_row_index():
    r = np.arange(D)
    m, p128 = r // P, r % P
    g_, cc = m // NCC, m % NCC
    p256 = cc * P + p128
    lh, hd = p256 // HD, p256 % HD
    return hd * HEADS + (HPC * g_ + lh)


def make_in_maps(x, qw, kw, vw, ow, s=S):
    scale = 1.0 / np.sqrt(np.float32(D))
    qws = (qw * scale).astype(np.float32)
    ow_perm = np.ascontiguousarray(ow[_ow_row_index()])
    in_maps = []
    xTs = [np.ascontiguousarray(x[b, :s].T) for b in range(B)]
    for c in range(N_CORES):
        b, g = c // GROUPS, c % GROUPS
        cols = _col_index(g)
        in_maps.append({
            "xT": xTs[b],
            "wq": np.ascontiguousarray(qws[:, cols]),
            "wk": np.ascontiguousarray(kw[:, cols]),
            "wv": np.ascontiguousarray(vw[:, cols]),
            "wo": np.ascontiguousarray(ow_perm[:, g * CPC:(g + 1) * CPC]),
        })
    return in_maps


def assemble_output(results, s=S):
    out = np.empty((B, s, D), dtype=np.float32)
    for c in range(N_CORES):
        b, g = c // GROUPS, c % GROUPS
        oT = results[c]["outT"]  # [NCC, P, s]
        for occ in range(NCC):
            out[b, :, g * CPC + occ * P:(g * CPC + (occ + 1) * P)] = oT[occ].T
    return out


def run_on_hw(x, qw, kw, vw, ow, trace=False, **cfg_over):
    cfg = dict(DEFAULT_CFG)
    cfg.update(cfg_over)
    s = cfg["s"]
    nc = get_nc(**cfg)
    in_maps = make_in_maps(x, qw, kw, vw, ow, s=s)
    res = run_bass_kernel_spmd(nc, in_maps, core_ids=list(range(N_CORES)),
                               trace=trace)
    return assemble_output(res.results, s=s), res


def kernel(x, qw, kw, vw, ow):
    out, _ = run_on_hw(np.asarray(x, dtype=np.float32),
                       np.asarray(qw, dtype=np.float32),
                       np.asarray(kw, dtype=np.float32),
                       np.asarray(vw, dtype=np.float32),
                       np.asarray(ow, dtype=np.float32))
    return out



# revision 14
# speedup vs baseline: 1.1643x; 1.1643x over previous
"""Multi-head attention (dense_transformer) on 8 TRN2 NeuronCores.

Sharding: 2-way data parallel over batch x 4-way tensor parallel over heads.
Core c handles batch b=c//4 and heads {4g..4g+3} where g=c%4 (4 heads, 256
channels per core; channels of head h are qw columns {hd*16+h}).

Architecture (v2, "transposed scores"):
  phase 1: Q^T/K^T projections in [ch, s] layout via 3-term bf16 splits
           (pseudo-fp32, needed because softmax here is a near-argmax: score
           std ~256, so score errors flip the max). V is projected DIRECTLY
           into [s, ch] layout with single-pass f32r matmuls (V error is
           linear in the output -> 11-bit mantissa is plenty), with a ones
           column appended per head so AV also produces the softmax sums.
  phase 2: scores computed TRANSPOSED: scT[k,q] = K^T-chunk (stationary) x
           Q^T (moving), 3-term bf16. Per 512-wide q block: drain chunks to
           SBUF, running column-max on gpsimd (Pool engine, otherwise idle),
           one partition_all_reduce(max) -> bias replicated on all
           partitions, DVE subtract + ACT exp -> expT bf16, then
           AV = Vhat-chunk (stationary [128,65]) x expT (moving) accumulates
           O^T[ch,q] AND l[q] in PSUM with no transposes at all.
           Normalize = reciprocal of l + gpsimd partition_broadcast + the
           PSUM drain is a tensor_tensor multiply.
  phase 3: AllGather O^T across the 4 cores of the batch -> out-proj column
           slice (out^T = ow_perm^T @ merged^T, f32r) -> DMA out.

vs v1: no PE transposes (was 576 matmuls / ~97us), V projection 3x cheaper,
no separate normalize pass, no exp accum; PE stream is denser so it holds
the 2.4GHz p-state instead of 1.2GHz.
"""
import sys

sys.path.insert(0, "/opt/trn_rl_repo")

import numpy as np

import concourse.bass as bass
import concourse.mybir as mybir
import concourse.tile as tile
from concourse import bacc
from concourse import bass_isa
from concourse.bass_utils import run_bass_kernel_spmd

# ---- problem constants (hardcoded per harness contract) ----
B, S, D, HEADS = 2, 2048, 1024, 16
N_CORES = 8
GROUPS = 4                 # head-groups == cores per batch
HPC = HEADS // GROUPS      # heads per core (4)
HD = D // HEADS            # 64
CPC = HPC * HD             # channels per core (256)
P = 128
NCC = CPC // P             # col chunks per core (2)
DCH = D // P               # contraction chunks (8)
QB = 512                   # q block width (1 PSUM bank of f32)

f32 = mybir.dt.float32
f32r = mybir.dt.float32r
bf16 = mybir.dt.bfloat16

AX = mybir.AxisListType
EXP = mybir.ActivationFunctionType.Exp
MAXOP = mybir.AluOpType.max
SUB = mybir.AluOpType.subtract
MULT = mybir.AluOpType.mult

DEFAULT_CFG = dict(s=S)


def make_maskT(nc, maskT, mask_val=-1e10):
    """maskT[k, q] = 0 if q >= k else mask_val (transposed causal)."""
    sq = maskT.shape[0]
    nc.gpsimd.memset(maskT, mask_val)
    nc.gpsimd.affine_select(
        out=maskT,
        in_=maskT,
        compare_op=mybir.AluOpType.is_gt,
        fill=0.0,
        base=0,
        # keep mask_val where (k - q) > 0, else fill 0
        pattern=[[-1, sq]],
        channel_multiplier=1,
    )


def build_nc(s=S, dbg=False):
    assert s % QB == 0
    NQB = s // QB            # 512-wide q blocks
    NKC = s // P             # 128-wide k chunks
    KPB = QB // P            # k chunks per q block on the diagonal (4)
    VW = 65                  # V channels per (hp,h2) incl the ones column
    NH2 = NCC * 2            # head slots per core (4)

    nc = bacc.Bacc("TRN2", target_bir_lowering=False, debug=False,
                   num_devices=N_CORES)
    xT = nc.dram_tensor("xT", [D, s], f32, kind="ExternalInput").ap()
    wq = nc.dram_tensor("wq", [D, CPC], f32, kind="ExternalInput").ap()
    wk = nc.dram_tensor("wk", [D, CPC], f32, kind="ExternalInput").ap()
    wv = nc.dram_tensor("wv", [D, CPC], f32r, kind="ExternalInput").ap()
    wo = nc.dram_tensor("wo", [D, CPC], f32, kind="ExternalInput").ap()
    outT = nc.dram_tensor("outT", [NCC, P, s], f32, kind="ExternalOutput").ap()

    with tile.TileContext(nc) as tc:
        with (
            tc.tile_pool(name="cpool", bufs=1) as cpool,
            tc.tile_pool(name="wpool", bufs=1) as wpool,
            tc.tile_pool(name="big", bufs=1) as big,
            tc.tile_pool(name="stat", bufs=2) as stat,
            tc.tile_pool(name="ms", bufs=3) as ms,
            tc.tile_pool(name="op", bufs=2) as op,
            tc.tile_pool(name="dram", bufs=1, space="DRAM") as dpool,
        ):
            NQB_ = s // QB
            ag_in = {qb: dpool.tile([P, NCC, QB], bf16, tag=f"agi{qb}",
                                    name=f"agi{qb}")
                     for qb in range(NQB_ - 1)}
            ag_out = {qb: dpool.tile([GROUPS, P, NCC, QB], bf16,
                                     tag=f"ago{qb}", name=f"ago{qb}")
                      for qb in range(NQB_ - 1)}
            ag_in_h = {(NQB_ - 1, hp): dpool.tile([P, QB], bf16,
                                                  tag=f"agih{hp}",
                                                  name=f"agih{hp}")
                       for hp in range(NCC)}
            ag_out_h = {(NQB_ - 1, hp): dpool.tile([GROUPS, P, QB], bf16,
                                                   tag=f"agoh{hp}",
                                                   name=f"agoh{hp}")
                        for hp in range(NCC)}

            maskT = cpool.tile([P, P], f32, tag="maskT")
            make_maskT(nc, maskT[:])

            woh = wpool.tile([P, DCH, CPC], bf16, tag="woh")
            wol = wpool.tile([P, DCH, CPC], bf16, tag="wol")
            wv_sb = wpool.tile([P, DCH, CPC], f32r, tag="wv")
            nc.sync.dma_start(wv_sb[:], wv.rearrange("(o p) c -> p o c", p=P))
            wsplit = {}
            for nm in ("q", "k"):
                wh = wpool.tile([P, DCH, CPC], bf16, tag=f"w{nm}h", name=f"w{nm}h")
                wl = wpool.tile([P, DCH, CPC], bf16, tag=f"w{nm}l", name=f"w{nm}l")
                wsplit[nm] = [wh, wl]
            with tc.tile_pool(name="wload", bufs=1) as wload:
                wf0 = wload.tile([P, DCH, CPC], f32, tag="wf", name="wf")
                nc.sync.dma_start(wf0[:], wo.rearrange("(o p) c -> p o c", p=P))
                nc.vector.tensor_copy(woh[:], wf0[:])
                nc.vector.tensor_tensor(wol[:], wf0[:], woh[:], SUB)
                for nm, wdr in (("q", wq), ("k", wk)):
                    wf = wload.tile([P, DCH, CPC], f32, tag="wf", name="wf")
                    nc.sync.dma_start(wf[:], wdr.rearrange("(o p) c -> p o c", p=P))
                    wh, wl = wsplit[nm]
                    nc.vector.tensor_copy(wh[:], wf[:])
                    nc.vector.tensor_tensor(wl[:], wf[:], wh[:], SUB)

            QTh = big.tile([P, NCC, s], bf16, tag="QTh")
            QTl = big.tile([P, NCC, s], bf16, tag="QTl")
            KTh = big.tile([P, NCC, s], bf16, tag="KTh")
            KTl = big.tile([P, NCC, s], bf16, tag="KTl")
            # Vhat[k, :]: 4 groups of 65 cols: 64 V channels + a ones col
            Vsb = big.tile([P, NKC, NH2 * VW], bf16, tag="Vsb")
            OT = big.tile([P, NCC, s], bf16, tag="OT")
            stage = big.tile([P, NKC, QB], f32, tag="stage")
            expT = big.tile([P, NKC, QB], bf16, tag="expT")

            for g in range(NH2):
                nc.gpsimd.memset(Vsb[:, :, g * VW + 64], 1.0)

            # ---------------- phase 1: projections ----------------
            with (
                tc.tile_pool(name="psp", bufs=1, space="PSUM") as psp,
                tc.tile_pool(name="psv", bufs=1, space="PSUM") as psv,
                tc.tile_pool(name="xs", bufs=5) as xs,
            ):
                for qb in range(NQB):
                    accs = {}
                    for nm in ("q", "k"):
                        for cc in range(NCC):
                            accs[nm, cc] = psp.tile([P, QB], f32,
                                                    tag=f"pp{nm}{cc}",
                                                    name=f"pp{nm}{cc}")
                    vacc = [psv.tile([P, CPC], f32, tag=f"pv{r}", name=f"pv{r}")
                            for r in range(KPB)]
                    for di in range(DCH):
                        xt = xs.tile([P, QB], f32, tag="xt", name="xt")
                        nc.sync.dma_start(
                            xt[:], xT[di * P:(di + 1) * P, qb * QB:(qb + 1) * QB])
                        xth = xs.tile([P, QB], bf16, tag="xth", name="xth")
                        xtl = xs.tile([P, QB], bf16, tag="xtl", name="xtl")
                        nc.vector.tensor_copy(xth[:], xt[:])
                        nc.vector.tensor_tensor(xtl[:], xt[:], xth[:], SUB)
                        xtr = xs.tile([P, QB], f32r, tag="xtr", name="xtr")
                        nc.any.tensor_copy(xtr[:], xt[:])
                        for nm in ("q", "k"):
                            wh, wl = wsplit[nm]
                            for cc in range(NCC):
                                csl = slice(cc * P, (cc + 1) * P)
                                terms = [(wh, xth), (wh, xtl), (wl, xth)]
                                for ti, (wt, xtt) in enumerate(terms):
                                    nc.tensor.matmul(
                                        accs[nm, cc][:], wt[:, di, csl], xtt[:],
                                        start=(di == 0 and ti == 0),
                                        stop=(di == DCH - 1 and ti == len(terms) - 1))
                        for r in range(KPB):
                            nc.tensor.matmul(
                                vacc[r][:], xtr[:, r * P:(r + 1) * P],
                                wv_sb[:, di, :],
                                start=(di == 0), stop=(di == DCH - 1))
                    sl = slice(qb * QB, (qb + 1) * QB)
                    for cc in range(NCC):
                        for hi_t, lo_t, ps in ((QTh, QTl, accs["q", cc]),
                                               (KTh, KTl, accs["k", cc])):
                            nc.any.tensor_copy(hi_t[:, cc, sl], ps[:])
                            nc.vector.tensor_tensor(lo_t[:, cc, sl], ps[:],
                                                    hi_t[:, cc, sl], SUB)
                    for r in range(KPB):
                        ki = qb * KPB + r
                        # strided dest: 4 groups of 64 V channels (skip ones col)
                        dst = Vsb[:, ki].rearrange("p (g w) -> p g w", w=VW)[:, :, 0:64]
                        nc.any.tensor_copy(dst, vacc[r][:])

            # ---------------- phase 2 + 3, software-pipelined ----------------
            with (
                tc.tile_pool(name="pssc", bufs=4, space="PSUM") as pssc,
                tc.tile_pool(name="psot", bufs=2, space="PSUM") as psot,
                tc.tile_pool(name="pso", bufs=1, space="PSUM") as pso,
                tc.tile_pool(name="stgp", bufs=22) as stgp,
                tc.tile_pool(name="expp", bufs=8) as expp,
            ):
                def phase3_block(j):
                    """out-proj for q block j (consumes that block's gather)."""
                    qsl3 = slice(j * QB, (j + 1) * QB)
                    accs = [pso.tile([P, QB], f32, tag=f"po{occ}",
                                     name=f"po{occ}")
                            for occ in range(NCC)]
                    last = (j == NQB - 1)
                    order = (sorted(range(DCH), key=lambda m: (m % NCC, m // NCC))
                             if last else list(range(DCH)))
                    for i, mch in enumerate(order):
                        g_, cc_ = mch // NCC, mch % NCC
                        mt = ms.tile([P, QB], bf16, tag="mt", name="mt")
                        if last:
                            nc.sync.dma_start(mt[:], ag_out_h[j, cc_][g_, :, :])
                        else:
                            nc.sync.dma_start(mt[:], ag_out[j][g_, :, cc_, :])
                        for occ in range(NCC):
                            for wi, wt in enumerate((woh, wol)):
                                nc.tensor.matmul(
                                    accs[occ][:], wt[:, mch, occ * P:(occ + 1) * P],
                                    mt[:], start=(i == 0 and wi == 0),
                                    stop=(i == DCH - 1 and wi == 1))
                    for occ in range(NCC):
                        oo = op.tile([P, QB], f32, tag="oo", name="oo")
                        nc.any.tensor_copy(oo[:], accs[occ][:])
                        nc.sync.dma_start(outT[occ, :, qsl3], oo[:])

                def _gather(inp, outp):
                    nc.gpsimd.collective_compute(
                        "AllGather", mybir.AluOpType.bypass,
                        replica_groups=[[0, 1, 2, 3], [4, 5, 6, 7]],
                        ins=[inp], outs=[outp],
                    )

                def passA(qb, hp, h2):
                    hsl = slice(h2 * 64, (h2 + 1) * 64)
                    nkc = qb * KPB + KPB
                    rm = stat.tile([P, QB], f32, tag="rm", name="rm")
                    nc.gpsimd.memset(rm[:], -3e38)
                    sts = []
                    for kc in range(nkc):
                        diag = kc - qb * KPB
                        off = max(0, diag) * P
                        psc = pssc.tile([P, QB], f32, tag="psc", name="psc")
                        ksl = slice(kc * P, (kc + 1) * P)
                        mvsl = slice(qb * QB + off, (qb + 1) * QB)
                        terms = ((KTh, QTh), (KTh, QTl), (KTl, QTh))
                        for ti, (kt, qt) in enumerate(terms):
                            nc.tensor.matmul(
                                psc[:, off:], kt[hsl, hp, ksl],
                                qt[hsl, hp, mvsl],
                                start=(ti == 0), stop=(ti == 2))
                        stg = stgp.tile([P, QB], f32, tag="stg", name="stg")
                        nc.any.tensor_copy(stg[:, off:], psc[:, off:])
                        if diag >= 0:
                            nc.vector.tensor_tensor(
                                stg[:, off:off + P], stg[:, off:off + P],
                                maskT[:], mybir.AluOpType.add)
                        nc.vector.tensor_tensor(rm[:, off:], rm[:, off:],
                                                stg[:, off:], MAXOP)
                        sts.append((stg, off))
                    mrep = stat.tile([P, QB], f32, tag="mrep", name="mrep")
                    nc.gpsimd.partition_all_reduce(
                        mrep[:], rm[:], P, bass_isa.ReduceOp.max)
                    nc.vector.tensor_scalar_add(mrep[:], mrep[:], 6.0)
                    return (qb, hp, h2, sts, mrep)

                def passB(st):
                    qb, hp, h2, sts, mrep = st
                    qsl = slice(qb * QB, (qb + 1) * QB)
                    hsl = slice(h2 * 64, (h2 + 1) * 64)
                    nkc = len(sts)
                    otp = psot.tile([VW, QB], f32, tag="otp", name="otp")
                    vg = slice((hp * 2 + h2) * VW, (hp * 2 + h2 + 1) * VW)
                    for kc, (stg, off) in enumerate(sts):
                        nc.vector.tensor_tensor(stg[:, off:], stg[:, off:],
                                                mrep[:, off:], SUB)
                        ex = expp.tile([P, QB], bf16, tag="ex", name="ex")
                        nc.scalar.activation(ex[:, off:], stg[:, off:], EXP)
                        nc.tensor.matmul(otp[:, off:], Vsb[:, kc, vg],
                                         ex[:, off:],
                                         start=(kc == 0), stop=(kc == nkc - 1))
                    rec = stat.tile([1, QB], f32, tag="rec", name="rec")
                    nc.vector.reciprocal(rec[:], otp[64:65, :])
                    recb = stat.tile([64, QB], f32, tag="recb", name="recb")
                    nc.gpsimd.partition_broadcast(recb[:], rec[:], 64)
                    nc.vector.tensor_tensor(OT[hsl, hp, qsl], otp[0:64, :],
                                            recb[:], MULT)
                    # fire gathers / interleaved out-proj on block boundaries
                    if h2 == 1:
                        if qb == NQB - 1:
                            nc.sync.dma_start(ag_in_h[qb, hp][:],
                                              OT[:, hp, qsl])
                            _gather(ag_in_h[qb, hp][:], ag_out_h[qb, hp][:])
                        elif hp == NCC - 1:
                            nc.sync.dma_start(ag_in[qb][:], OT[:, :, qsl])
                            _gather(ag_in[qb][:], ag_out[qb][:])
                        if hp == NCC - 1 and qb >= 2:
                            phase3_block(qb - 2)

                blocks = [(qb, hp, h2) for qb in range(NQB)
                          for hp in range(NCC) for h2 in range(2)]
                prev = None
                for blk in blocks:
                    cur = passA(*blk)
                    if prev is not None:
                        passB(prev)
                    prev = cur
                passB(prev)
                for j in range(max(0, NQB - 2), NQB):
                    phase3_block(j)

    nc.compile()
    return nc


_NC_CACHE = {}


def get_nc(**cfg):
    key = tuple(sorted(cfg.items()))
    if key not in _NC_CACHE:
        _NC_CACHE[key] = build_nc(**cfg)
    return _NC_CACHE[key]


def _col_index(g):
    p = np.arange(CPC)
    return (p % HD) * HEADS + (HPC * g + p // HD)


def _ow_row_index():
    r = np.arange(D)
    m, p128 = r // P, r % P
    g_, cc = m // NCC, m % NCC
    p256 = cc * P + p128
    lh, hd = p256 // HD, p256 % HD
    return hd * HEADS + (HPC * g_ + lh)


def make_in_maps(x, qw, kw, vw, ow, s=S):
    scale = 1.0 / np.sqrt(np.float32(D))
    qws = (qw * scale).astype(np.float32)
    ow_perm = np.ascontiguousarray(ow[_ow_row_index()])
    in_maps = []
    xTs = [np.ascontiguousarray(x[b, :s].T) for b in range(B)]
    for c in range(N_CORES):
        b, g = c // GROUPS, c % GROUPS
        cols = _col_index(g)
        in_maps.append({
            "xT": xTs[b],
            "wq": np.ascontiguousarray(qws[:, cols]),
            "wk": np.ascontiguousarray(kw[:, cols]),
            "wv": np.ascontiguousarray(vw[:, cols]),
            "wo": np.ascontiguousarray(ow_perm[:, g * CPC:(g + 1) * CPC]),
        })
    return in_maps


def assemble_output(results, s=S):
    out = np.empty((B, s, D), dtype=np.float32)
    for c in range(N_CORES):
        b, g = c // GROUPS, c % GROUPS
        oT = results[c]["outT"]  # [NCC, P, s]
        for occ in range(NCC):
            out[b, :, g * CPC + occ * P:(g * CPC + (occ + 1) * P)] = oT[occ].T
    return out


def run_on_hw(x, qw, kw, vw, ow, trace=False, **cfg_over):
    cfg = dict(DEFAULT_CFG)
    cfg.update(cfg_over)
    s = cfg["s"]
    nc = get_nc(**cfg)
    in_maps = make_in_maps(x, qw, kw, vw, ow, s=s)
    res = run_bass_kernel_spmd(nc, in_maps, core_ids=list(range(N_CORES)),
                               trace=trace)
    return assemble_output(res.results, s=s), res


def kernel(x, qw, kw, vw, ow):
    out, _ = run_on_hw(np.asarray(x, dtype=np.float32),
                       np.asarray(qw, dtype=np.float32),
                       np.asarray(kw, dtype=np.float32),
                       np.asarray(vw, dtype=np.float32),
                       np.asarray(ow, dtype=np.float32))
    return out


# revision 15
# speedup vs baseline: 1.1982x; 1.0291x over previous
"""Multi-head attention (dense_transformer) on 8 TRN2 NeuronCores.

Sharding: 2-way data parallel over batch x 4-way tensor parallel over heads.
Core c handles batch b=c//4 and heads {4g..4g+3} where g=c%4 (4 heads, 256
channels per core; channels of head h are qw columns {hd*16+h}).

Architecture (v2, "transposed scores"):
  phase 1: Q^T/K^T projections in [ch, s] layout via 3-term bf16 splits
           (pseudo-fp32, needed because softmax here is a near-argmax: score
           std ~256, so score errors flip the max). V is projected DIRECTLY
           into [s, ch] layout with single-pass f32r matmuls (V error is
           linear in the output -> 11-bit mantissa is plenty), with a ones
           column appended per head so AV also produces the softmax sums.
  phase 2: scores computed TRANSPOSED: scT[k,q] = K^T-chunk (stationary) x
           Q^T (moving), 3-term bf16. Per 512-wide q block: drain chunks to
           SBUF, running column-max on gpsimd (Pool engine, otherwise idle),
           one partition_all_reduce(max) -> bias replicated on all
           partitions, DVE subtract + ACT exp -> expT bf16, then
           AV = Vhat-chunk (stationary [128,65]) x expT (moving) accumulates
           O^T[ch,q] AND l[q] in PSUM with no transposes at all.
           Normalize = reciprocal of l + gpsimd partition_broadcast + the
           PSUM drain is a tensor_tensor multiply.
  phase 3: AllGather O^T across the 4 cores of the batch -> out-proj column
           slice (out^T = ow_perm^T @ merged^T, f32r) -> DMA out.

vs v1: no PE transposes (was 576 matmuls / ~97us), V projection 3x cheaper,
no separate normalize pass, no exp accum; PE stream is denser so it holds
the 2.4GHz p-state instead of 1.2GHz.
"""
import sys

sys.path.insert(0, "/opt/trn_rl_repo")

import numpy as np

import concourse.bass as bass
import concourse.mybir as mybir
import concourse.tile as tile
from concourse import bacc
from concourse import bass_isa
from concourse.bass_utils import run_bass_kernel_spmd

# ---- problem constants (hardcoded per harness contract) ----
B, S, D, HEADS = 2, 2048, 1024, 16
N_CORES = 8
GROUPS = 4                 # head-groups == cores per batch
HPC = HEADS // GROUPS      # heads per core (4)
HD = D // HEADS            # 64
CPC = HPC * HD             # channels per core (256)
P = 128
NCC = CPC // P             # col chunks per core (2)
DCH = D // P               # contraction chunks (8)
QB = 512                   # q block width (1 PSUM bank of f32)

f32 = mybir.dt.float32
f32r = mybir.dt.float32r
bf16 = mybir.dt.bfloat16

AX = mybir.AxisListType
EXP = mybir.ActivationFunctionType.Exp
MAXOP = mybir.AluOpType.max
SUB = mybir.AluOpType.subtract
MULT = mybir.AluOpType.mult
COPYF = mybir.ActivationFunctionType.Copy

DEFAULT_CFG = dict(s=S)


def make_maskT(nc, maskT, mask_val=-1e10):
    """maskT[k, q] = 0 if q >= k else mask_val (transposed causal)."""
    sq = maskT.shape[0]
    nc.gpsimd.memset(maskT, mask_val)
    nc.gpsimd.affine_select(
        out=maskT,
        in_=maskT,
        compare_op=mybir.AluOpType.is_gt,
        fill=0.0,
        base=0,
        # keep mask_val where (k - q) > 0, else fill 0
        pattern=[[-1, sq]],
        channel_multiplier=1,
    )


def build_nc(s=S, dbg=False):
    assert s % QB == 0
    NQB = s // QB            # 512-wide q blocks
    NKC = s // P             # 128-wide k chunks
    KPB = QB // P            # k chunks per q block on the diagonal (4)
    VW = 65                  # V channels per (hp,h2) incl the ones column
    NH2 = NCC * 2            # head slots per core (4)

    nc = bacc.Bacc("TRN2", target_bir_lowering=False, debug=False,
                   num_devices=N_CORES)
    xT = nc.dram_tensor("xT", [D, s], f32, kind="ExternalInput").ap()
    wq = nc.dram_tensor("wq", [D, CPC], f32, kind="ExternalInput").ap()
    wk = nc.dram_tensor("wk", [D, CPC], f32, kind="ExternalInput").ap()
    wv = nc.dram_tensor("wv", [D, CPC], f32r, kind="ExternalInput").ap()
    wo = nc.dram_tensor("wo", [D, CPC], f32, kind="ExternalInput").ap()
    outT = nc.dram_tensor("outT", [NCC, P, s], f32, kind="ExternalOutput").ap()

    with tile.TileContext(nc) as tc:
        with (
            tc.tile_pool(name="cpool", bufs=1) as cpool,
            tc.tile_pool(name="wpool", bufs=1) as wpool,
            tc.tile_pool(name="big", bufs=1) as big,
            tc.tile_pool(name="stat", bufs=2) as stat,
            tc.tile_pool(name="ms", bufs=3) as ms,
            tc.tile_pool(name="op", bufs=2) as op,
            tc.tile_pool(name="dram", bufs=1, space="DRAM") as dpool,
        ):
            NQB_ = s // QB
            ag_in = {qb: dpool.tile([P, NCC, QB], bf16, tag=f"agi{qb}",
                                    name=f"agi{qb}")
                     for qb in range(NQB_ - 1)}
            ag_out = {qb: dpool.tile([GROUPS, P, NCC, QB], bf16,
                                     tag=f"ago{qb}", name=f"ago{qb}")
                      for qb in range(NQB_ - 1)}
            ag_in_h = {(NQB_ - 1, hp): dpool.tile([P, QB], bf16,
                                                  tag=f"agih{hp}",
                                                  name=f"agih{hp}")
                       for hp in range(NCC)}
            ag_out_h = {(NQB_ - 1, hp): dpool.tile([GROUPS, P, QB], bf16,
                                                   tag=f"agoh{hp}",
                                                   name=f"agoh{hp}")
                        for hp in range(NCC)}

            maskT = cpool.tile([P, P], f32, tag="maskT")
            make_maskT(nc, maskT[:])

            woh = wpool.tile([P, DCH, CPC], bf16, tag="woh")
            wol = wpool.tile([P, DCH, CPC], bf16, tag="wol")
            wv_sb = wpool.tile([P, DCH, CPC], f32r, tag="wv")
            for di in range(DCH):
                nc.sync.dma_start(wv_sb[:, di, :], wv[di * P:(di + 1) * P, :])
            wsplit = {}
            for nm in ("q", "k"):
                wh = wpool.tile([P, DCH, CPC], bf16, tag=f"w{nm}h", name=f"w{nm}h")
                wl = wpool.tile([P, DCH, CPC], bf16, tag=f"w{nm}l", name=f"w{nm}l")
                wsplit[nm] = [wh, wl]
            with tc.tile_pool(name="wload", bufs=2) as wload:
                for nm, wdr in (("q", wq), ("k", wk), ("o", wo)):
                    wf = wload.tile([P, DCH, CPC], f32, tag="wf", name="wf")
                    for di in range(DCH):
                        nc.sync.dma_start(wf[:, di, :],
                                          wdr[di * P:(di + 1) * P, :])
                    wh, wl = ((woh, wol) if nm == "o" else wsplit[nm])
                    nc.vector.tensor_copy(wh[:], wf[:])
                    nc.vector.tensor_tensor(wl[:], wf[:], wh[:], SUB)

            QTh = big.tile([P, NCC, s], bf16, tag="QTh")
            QTl = big.tile([P, NCC, s], bf16, tag="QTl")
            KTh = big.tile([P, NCC, s], bf16, tag="KTh")
            KTl = big.tile([P, NCC, s], bf16, tag="KTl")
            # Vhat[k, :]: 4 groups of 65 cols: 64 V channels + a ones col
            Vsb = big.tile([P, NKC, NH2 * VW], bf16, tag="Vsb")
            OT = big.tile([P, NCC, s], bf16, tag="OT")
            stage = big.tile([P, NKC, QB], f32, tag="stage")
            expT = big.tile([P, NKC, QB], bf16, tag="expT")

            for g in range(NH2):
                nc.gpsimd.memset(Vsb[:, :, g * VW + 64], 1.0)

            # ---------------- phase 1: projections ----------------
            with (
                tc.tile_pool(name="psp", bufs=1, space="PSUM") as psp,
                tc.tile_pool(name="psv", bufs=1, space="PSUM") as psv,
                tc.tile_pool(name="xs", bufs=5) as xs,
            ):
                for qb in range(NQB):
                    accs = {}
                    for nm in ("q", "k"):
                        for cc in range(NCC):
                            accs[nm, cc] = psp.tile([P, QB], f32,
                                                    tag=f"pp{nm}{cc}",
                                                    name=f"pp{nm}{cc}")
                    vacc = [psv.tile([P, CPC], f32, tag=f"pv{r}", name=f"pv{r}")
                            for r in range(KPB)]
                    for di in range(DCH):
                        xt = xs.tile([P, QB], f32, tag="xt", name="xt")
                        nc.sync.dma_start(
                            xt[:], xT[di * P:(di + 1) * P, qb * QB:(qb + 1) * QB])
                        xth = xs.tile([P, QB], bf16, tag="xth", name="xth")
                        xtl = xs.tile([P, QB], bf16, tag="xtl", name="xtl")
                        nc.vector.tensor_copy(xth[:], xt[:])
                        nc.vector.tensor_tensor(xtl[:], xt[:], xth[:], SUB)
                        xtr = xs.tile([P, QB], f32r, tag="xtr", name="xtr")
                        nc.any.tensor_copy(xtr[:], xt[:])
                        for nm in ("q", "k"):
                            wh, wl = wsplit[nm]
                            for cc in range(NCC):
                                csl = slice(cc * P, (cc + 1) * P)
                                terms = [(wh, xth), (wh, xtl), (wl, xth)]
                                for ti, (wt, xtt) in enumerate(terms):
                                    nc.tensor.matmul(
                                        accs[nm, cc][:], wt[:, di, csl], xtt[:],
                                        start=(di == 0 and ti == 0),
                                        stop=(di == DCH - 1 and ti == len(terms) - 1))
                        for r in range(KPB):
                            nc.tensor.matmul(
                                vacc[r][:], xtr[:, r * P:(r + 1) * P],
                                wv_sb[:, di, :],
                                start=(di == 0), stop=(di == DCH - 1))
                    sl = slice(qb * QB, (qb + 1) * QB)
                    for cc in range(NCC):
                        for hi_t, lo_t, ps in ((QTh, QTl, accs["q", cc]),
                                               (KTh, KTl, accs["k", cc])):
                            nc.any.tensor_copy(hi_t[:, cc, sl], ps[:])
                            nc.vector.tensor_tensor(lo_t[:, cc, sl], ps[:],
                                                    hi_t[:, cc, sl], SUB)
                    for r in range(KPB):
                        ki = qb * KPB + r
                        # strided dest: 4 groups of 64 V channels (skip ones col)
                        dst = Vsb[:, ki].rearrange("p (g w) -> p g w", w=VW)[:, :, 0:64]
                        nc.any.tensor_copy(dst, vacc[r][:])

            # ---------------- phase 2 + 3, software-pipelined ----------------
            with (
                tc.tile_pool(name="pssc", bufs=4, space="PSUM") as pssc,
                tc.tile_pool(name="psot", bufs=2, space="PSUM") as psot,
                tc.tile_pool(name="pso", bufs=1, space="PSUM") as pso,
                tc.tile_pool(name="stgp", bufs=22) as stgp,
                tc.tile_pool(name="expp", bufs=8) as expp,
            ):
                def phase3_block(j):
                    """out-proj for q block j (consumes that block's gather)."""
                    qsl3 = slice(j * QB, (j + 1) * QB)
                    accs = [pso.tile([P, QB], f32, tag=f"po{occ}",
                                     name=f"po{occ}")
                            for occ in range(NCC)]
                    last = (j == NQB - 1)
                    order = (sorted(range(DCH), key=lambda m: (m % NCC, m // NCC))
                             if last else list(range(DCH)))
                    for i, mch in enumerate(order):
                        g_, cc_ = mch // NCC, mch % NCC
                        mt = ms.tile([P, QB], bf16, tag="mt", name="mt")
                        if last:
                            nc.sync.dma_start(mt[:], ag_out_h[j, cc_][g_, :, :])
                        else:
                            nc.sync.dma_start(mt[:], ag_out[j][g_, :, cc_, :])
                        for occ in range(NCC):
                            for wi, wt in enumerate((woh, wol)):
                                nc.tensor.matmul(
                                    accs[occ][:], wt[:, mch, occ * P:(occ + 1) * P],
                                    mt[:], start=(i == 0 and wi == 0),
                                    stop=(i == DCH - 1 and wi == 1))
                    for occ in range(NCC):
                        oo = op.tile([P, QB], f32, tag="oo", name="oo")
                        nc.any.tensor_copy(oo[:], accs[occ][:])
                        nc.sync.dma_start(outT[occ, :, qsl3], oo[:])

                def _gather(inp, outp):
                    nc.gpsimd.collective_compute(
                        "AllGather", mybir.AluOpType.bypass,
                        replica_groups=[[0, 1, 2, 3], [4, 5, 6, 7]],
                        ins=[inp], outs=[outp],
                    )

                def passA(qb, hp, h2):
                    hsl = slice(h2 * 64, (h2 + 1) * 64)
                    nkc = qb * KPB + KPB
                    rm = stat.tile([P, QB], f32, tag="rm", name="rm")
                    nc.gpsimd.memset(rm[:], -3e38)
                    sts = []
                    for kc in range(nkc):
                        diag = kc - qb * KPB
                        off = max(0, diag) * P
                        psc = pssc.tile([P, QB], f32, tag="psc", name="psc")
                        ksl = slice(kc * P, (kc + 1) * P)
                        mvsl = slice(qb * QB + off, (qb + 1) * QB)
                        terms = ((KTh, QTh), (KTh, QTl), (KTl, QTh))
                        for ti, (kt, qt) in enumerate(terms):
                            nc.tensor.matmul(
                                psc[:, off:], kt[hsl, hp, ksl],
                                qt[hsl, hp, mvsl],
                                start=(ti == 0), stop=(ti == 2))
                        stg = stgp.tile([P, QB], f32, tag="stg", name="stg")
                        nc.scalar.activation(stg[:, off:], psc[:, off:], COPYF)
                        if diag >= 0:
                            nc.vector.tensor_tensor(
                                stg[:, off:off + P], stg[:, off:off + P],
                                maskT[:], mybir.AluOpType.add)
                        nc.vector.tensor_tensor(rm[:, off:], rm[:, off:],
                                                stg[:, off:], MAXOP)
                        sts.append((stg, off))
                    mrep = stat.tile([P, QB], f32, tag="mrep", name="mrep")
                    nc.gpsimd.partition_all_reduce(
                        mrep[:], rm[:], P, bass_isa.ReduceOp.max)
                    nc.vector.tensor_scalar_add(mrep[:], mrep[:], 6.0)
                    return (qb, hp, h2, sts, mrep)

                def passB(st):
                    qb, hp, h2, sts, mrep = st
                    qsl = slice(qb * QB, (qb + 1) * QB)
                    hsl = slice(h2 * 64, (h2 + 1) * 64)
                    nkc = len(sts)
                    otp = psot.tile([VW, QB], f32, tag="otp", name="otp")
                    vg = slice((hp * 2 + h2) * VW, (hp * 2 + h2 + 1) * VW)
                    for kc, (stg, off) in enumerate(sts):
                        nc.vector.tensor_tensor(stg[:, off:], stg[:, off:],
                                                mrep[:, off:], SUB)
                        ex = expp.tile([P, QB], bf16, tag="ex", name="ex")
                        nc.scalar.activation(ex[:, off:], stg[:, off:], EXP)
                        nc.tensor.matmul(otp[:, off:], Vsb[:, kc, vg],
                                         ex[:, off:],
                                         start=(kc == 0), stop=(kc == nkc - 1))
                    rec = stat.tile([1, QB], f32, tag="rec", name="rec")
                    nc.vector.reciprocal(rec[:], otp[64:65, :])
                    recb = stat.tile([64, QB], f32, tag="recb", name="recb")
                    nc.gpsimd.partition_broadcast(recb[:], rec[:], 64)
                    nc.vector.tensor_tensor(OT[hsl, hp, qsl], otp[0:64, :],
                                            recb[:], MULT)
                    # fire gathers / interleaved out-proj on block boundaries
                    if h2 == 1:
                        if qb == NQB - 1:
                            nc.sync.dma_start(ag_in_h[qb, hp][:],
                                              OT[:, hp, qsl])
                            _gather(ag_in_h[qb, hp][:], ag_out_h[qb, hp][:])
                        elif hp == NCC - 1:
                            nc.sync.dma_start(ag_in[qb][:], OT[:, :, qsl])
                            _gather(ag_in[qb][:], ag_out[qb][:])
                        if hp == NCC - 1 and qb >= 2:
                            phase3_block(qb - 2)

                blocks = [(qb, hp, h2) for qb in range(NQB)
                          for hp in range(NCC) for h2 in range(2)]
                prev = None
                for blk in blocks:
                    cur = passA(*blk)
                    if prev is not None:
                        passB(prev)
                    prev = cur
                passB(prev)
                for j in range(max(0, NQB - 2), NQB):
                    phase3_block(j)

    nc.compile()
    return nc


_NC_CACHE = {}


def get_nc(**cfg):
    key = tuple(sorted(cfg.items()))
    if key not in _NC_CACHE:
        _NC_CACHE[key] = build_nc(**cfg)
    return _NC_CACHE[key]


def _col_index(g):
    p = np.arange(CPC)
    return (p % HD) * HEADS + (HPC * g + p // HD)


def _ow_row_index():
    r = np.arange(D)
    m, p128 = r // P, r % P
    g_, cc = m // NCC, m % NCC
    p256 = cc * P + p128
    lh, hd = p256 // HD, p256 % HD
    return hd * HEADS + (HPC * g_ + lh)


def make_in_maps(x, qw, kw, vw, ow, s=S):
    scale = 1.0 / np.sqrt(np.float32(D))
    qws = (qw * scale).astype(np.float32)
    ow_perm = np.ascontiguousarray(ow[_ow_row_index()])
    in_maps = []
    xTs = [np.ascontiguousarray(x[b, :s].T) for b in range(B)]
    for c in range(N_CORES):
        b, g = c // GROUPS, c % GROUPS
        cols = _col_index(g)
        in_maps.append({
            "xT": xTs[b],
            "wq": np.ascontiguousarray(qws[:, cols]),
            "wk": np.ascontiguousarray(kw[:, cols]),
            "wv": np.ascontiguousarray(vw[:, cols]),
            "wo": np.ascontiguousarray(ow_perm[:, g * CPC:(g + 1) * CPC]),
        })
    return in_maps


def assemble_output(results, s=S):
    out = np.empty((B, s, D), dtype=np.float32)
    for c in range(N_CORES):
        b, g = c // GROUPS, c % GROUPS
        oT = results[c]["outT"]  # [NCC, P, s]
        for occ in range(NCC):
            out[b, :, g * CPC + occ * P:(g * CPC + (occ + 1) * P)] = oT[occ].T
    return out


def run_on_hw(x, qw, kw, vw, ow, trace=False, **cfg_over):
    cfg = dict(DEFAULT_CFG)
    cfg.update(cfg_over)
    s = cfg["s"]
    nc = get_nc(**cfg)
    in_maps = make_in_maps(x, qw, kw, vw, ow, s=s)
    res = run_bass_kernel_spmd(nc, in_maps, core_ids=list(range(N_CORES)),
                               trace=trace)
    return assemble_output(res.results, s=s), res


def kernel(x, qw, kw, vw, ow):
    out, _ = run_on_hw(np.asarray(x, dtype=np.float32),
                       np.asarray(qw, dtype=np.float32),
                       np.asarray(kw, dtype=np.float32),
                       np.asarray(vw, dtype=np.float32),
                       np.asarray(ow, dtype=np.float32))
    return out
